# revision 6
# baseline (speedup 1.0000x reference)
"""LogEig Trainium2 kernel v2i4: X = log(P) for SPD P via composite polynomial.

Math: every matmul-built tensor is a polynomial of P, so the kernel's exact
scalar action on eigenvalues is a designed composite function:
    d_0 = lam;  d_{i+1} = d_i + sigma_i d_i^2   (K=8 contraction steps)
    log(lam) ~= c0 + sum_i c_i d_i + cf2 * d_K^2    (global least-squares fit,
                                                     total rel err ~1.4e-3)
Per step: one batched 64x64 product (PE, quadrant-packed pairs) + one fused
AXPY (DVE) + one accumulation AXPY (Pool). 9 products per matrix total.
Batch of 8192 matrices sharded over 8 NeuronCores (1024 each).
"""

import numpy as np

import concourse.bass as bass
import concourse.mybir as mybir
from concourse import bacc
from concourse.bass import ds
from concourse.bass_utils import run_bass_kernel_spmd
from concourse.tile import TileContext

F32 = mybir.dt.float32
ALU = mybir.AluOpType

# ---------------- designed constants (see design2.py) ----------------
SIGMAS = [-1.075177135e-01, -2.867541926e-01, -7.649643581e-01,
          -2.041943548e+00, -5.459703523e+00, -1.466292403e+01,
          -3.984730093e+01, -1.117281157e+02]
C0 = -8.102624854e+00
C_D = [2.196022600e-01, 4.297179445e-01, 1.210738248e+00, 3.196899612e+00,
       8.584713458e+00, 2.307315480e+01, 6.323317755e+01, 1.819681532e+02,
       1.337874966e+03]
CF2EFF = -5.832597604e+02 * 4.469124630e+02   # c[f2] * tau2

K_STEPS = 8
N_MAT = 1024                  # matrices per core
BLK = 16                      # matrices per tile (2 decks x 8 pairs)
INTERLEAVE = 4


def _mm_sq(nc, psum, W):
    """psum = W @ W per 64x64 matrix; 8 pairs x 2 deck quadrants."""
    for p in range(BLK // 2):
        cs = ds(64 * p, 64)
        nc.tensor.matmul(psum[0:64, cs], W[0:64, cs], W[0:64, cs],
                         start=True, stop=True, tile_position=(0, 0))
        nc.tensor.matmul(psum[64:128, cs], W[64:128, cs], W[64:128, cs],
                         start=True, stop=True, tile_position=(64, 64))


def _emit_block(nc, pool, pspool, base, P_d, O_d, IC, par):
    D = pool.tile([128, 512], F32, tag=f"D0_{par}")
    nc.sync.dma_start(D[0:64, :], P_d[ds(base, 8)].transpose([1, 0, 2]))
    nc.sync.dma_start(D[64:128, :], P_d[ds(base + 8, 8)].transpose([1, 0, 2]))

    ACC = pool.tile([128, 512], F32, tag=f"ACC_{par}")
    # ACC = c_d0 * D0 + C0 * I   (IC holds C0 on block diagonals)
    nc.vector.scalar_tensor_tensor(ACC, D, float(C_D[0]), IC,
                                   ALU.mult, ALU.add)
    for i in range(K_STEPS):
        ps = pspool.tile([128, 512], F32, tag=f"ps{par}")
        _mm_sq(nc, ps, D)
        Dn = pool.tile([128, 512], F32, tag=f"D{(i + 1) % 2}_{par}")
        nc.vector.scalar_tensor_tensor(Dn, ps, float(SIGMAS[i]), D,
                                       ALU.mult, ALU.add)
        D = Dn
        nc.vector.scalar_tensor_tensor(ACC, D, float(C_D[i + 1]), ACC,
                                       ALU.mult, ALU.add)
    psf = pspool.tile([128, 512], F32, tag=f"ps{par}")
    _mm_sq(nc, psf, D)
    OUT = pool.tile([128, 512], F32, tag=f"OUT_{par}")
    nc.vector.scalar_tensor_tensor(OUT, psf, float(CF2EFF), ACC,
                                   ALU.mult, ALU.add)
    nc.sync.dma_start(O_d[ds(base, 8)].transpose([1, 0, 2]), OUT[0:64, :])
    nc.sync.dma_start(O_d[ds(base + 8, 8)].transpose([1, 0, 2]), OUT[64:128, :])


def build_nc(n_mat=N_MAT, unroll=False):
    nc = bacc.Bacc("TRN2", target_bir_lowering=False, debug=False,
                   num_devices=8)
    P_d = nc.dram_tensor("P", [n_mat, 64, 64], F32, kind="ExternalInput").ap()
    O_d = nc.dram_tensor("OUT", [n_mat, 64, 64], F32, kind="ExternalOutput").ap()
    IC_d = nc.dram_tensor("IC", [128, 512], F32, kind="ExternalInput").ap()
    with TileContext(nc) as tc:
        with (
            tc.tile_pool(name="consts", bufs=1) as cpool,
            tc.tile_pool(name="work", bufs=2) as pool,
            tc.tile_pool(name="psum", bufs=2, space=bass.MemorySpace.PSUM) as pspool,
        ):
            IC = cpool.tile([128, 512], F32)
            nc.sync.dma_start(IC[:], IC_d)
            step = BLK * INTERLEAVE
            if unroll:
                for m0 in range(0, n_mat, step):
                    for par in range(INTERLEAVE):
                        _emit_block(nc, pool, pspool, m0 + par * BLK,
                                    P_d, O_d, IC, par)
            else:
                with tc.For_i(0, n_mat, step) as m0:
                    for par in range(INTERLEAVE):
                        _emit_block(nc, pool, pspool, m0 + par * BLK,
                                    P_d, O_d, IC, par)
    nc.compile()
    return nc


def _ic_const():
    ic = np.zeros((128, 512), np.float32)
    for p in range(128):
        for k in range(8):
            ic[p, 64 * k + (p % 64)] = C0
    return ic


def host_constants():
    """Per-core constant input tensors (besides the sharded P)."""
    return {"IC": _ic_const()}


_NC_CACHE = {}


def kernel(P: np.ndarray) -> np.ndarray:
    P = np.ascontiguousarray(np.asarray(P), dtype=np.float32)
    B, H, N, _ = P.shape            # (1024, 8, 64, 64)
    flat = P.reshape(-1, N, N)      # 8192 matrices
    n_cores = 8
    per = flat.shape[0] // n_cores  # 1024
    if "nc" not in _NC_CACHE:
        _NC_CACHE["nc"] = build_nc()
    nc = _NC_CACHE["nc"]
    ic = _ic_const()
    in_maps = [
        {"P": np.ascontiguousarray(flat[c * per:(c + 1) * per]), "IC": ic}
        for c in range(n_cores)
    ]
    res = run_bass_kernel_spmd(nc, in_maps, core_ids=list(range(n_cores)))
    out = np.concatenate([r["OUT"] for r in res.results], axis=0)
    return out.reshape(B, H, N, N).astype(np.float32)
